# revision 23
# baseline (speedup 1.0000x reference)
"""Multi-head attention (B=8, N=1024, C=1024, H=16) on 8 TRN2 NeuronCores.

Sharding: batch-parallel — core c computes batch c end-to-end (12.9 GFLOP
per core, no collectives, output is a concat).

Design (v5):
  - All matmul operands are bf16 (f32 PSUM accumulation); x is transposed on
    the HOST (input xT [C, N] per core), weights are packed on the HOST into
    per-head-pair contiguous layouts so every weight DMA is a contiguous
    >=1KB-per-line read.
  - Weights (wq/wk/wv/wp) are DMA'd once (rep 0) and stay resident in SBUF
    across reps; per-rep DMA is only x in (2MB) and y out (2MB bf16).
  - qkv(0) accumulates chunk-major into 4 PSUM banks so the PE paces with
    x-chunk DMA arrival instead of stalling for the full x transfer.
  - Exp processes 2-bank PSUM tiles [128, 2, 512]: one Act instruction per
    two S^T chunks.
  - Software pipeline over head-pairs hp: qkv(hp+1) runs on the PE while
    exp(hp) drains on the Activation engine; S(u) matmul passes interleave
    with attn@v of the previous unit.
  - Queues: SP = x + wp + bias (head work), Act = wq/wk/wv (resident-weight
    loads), Pool = inv-broadcast roundtrips + y out (tail work) — so rep
    r+1's x prefetches during rep r's projection phase.
  - The two projection-gating units of hp7 normalize via a PE ones-broadcast
    of 1/rowsum instead of the DRAM-roundtrip DMA broadcast (shorter
    critical path into the projection).
  - y is returned bf16 (cast to f32 on host), halving output DMA.

Per-core algorithm (xT [C, N], packed weights):
  q^T,k^T[hp] = (W_q|W_k cols).T @ xT  -> qk[hp] [128, 2, N] (2 heads/tile)
  v = xT.T @ W_v                       -> v_flat [128, 8, 16, 65] with ones
                                          column (softmax row sums for free)
  per unit (head, m-block): S^T chunk-pair -> exp [128,2,512] -> P' bf16
  out'^T [65, 512] accumulated on PE; col 64 = row sums; normalize via
  reciprocal + partition-broadcast; y = oT.T @ W_proj + bias (bf16 out).
"""

import contextlib
import numpy as np

B, N, C, H, D = 8, 1024, 1024, 16, 64
HP = H // 2
SCALE = D ** -0.5
NCORES = 8
PCHUNKS = C // 128
TB = 512

_cached = {}


def _split_excess_waits(nc, max_waits=1):
    """walrus codegen limit: several lowered instruction structs (4-byte
    self-loading matmul S3_LW, drain CTRL_NO) carry only one sync-wait slot.
    Hoist excess waits onto InstEventSemaphore (2 waits each) just before
    the instruction on the same engine."""
    import concourse.mybir as mybir

    for func in nc.m.functions:
        for bb in func.blocks:
            insts = list(bb.instructions)
            out = []
            changed = False
            for inst in insts:
                si = inst.sync_info
                if (
                    si is not None
                    and not isinstance(inst, mybir.InstEventSemaphore)
                    and len(si.on_wait) > max_waits
                ):
                    waits = list(si.on_wait)
                    keep, excess = waits[:max_waits], waits[max_waits:]
                    for j in range(0, len(excess), 2):
                        ev = mybir.InstEventSemaphore(
                            name=nc.get_next_instruction_name(),
                            engine=inst.engine,
                            ins=[],
                            outs=[],
                            sync_info=mybir.SyncInfo(
                                on_wait=excess[j : j + 2], on_update=[]
                            ),
                        )
                        nc.register_instruction(ev)
                        out.append(ev)
                    si.on_wait = keep
                    inst.sync_info = si
                    changed = True
                out.append(inst)
            if changed:
                bb.instructions = out


def _build(n_rep=1):
    import concourse.bass as bass
    import concourse.mybir as mybir
    import concourse.tile as tile

    f32 = mybir.dt.float32
    bf16 = mybir.dt.bfloat16
    Exp = mybir.ActivationFunctionType.Exp

    nc = bass.Bass()
    xT = nc.declare_dram_parameter("xT", [C, N], bf16, isOutput=False)
    # host-packed weights: contiguous per-DMA layouts
    wq_pk = nc.declare_dram_parameter("wq_pk", [HP, 128, PCHUNKS, 128], bf16, isOutput=False)
    wk_pk = nc.declare_dram_parameter("wk_pk", [HP, 128, PCHUNKS, 128], bf16, isOutput=False)
    wv_pk = nc.declare_dram_parameter("wv_pk", [2, 128, PCHUNKS, TB], bf16, isOutput=False)
    wp_pk = nc.declare_dram_parameter("wp_pk", [2, 128, PCHUNKS, TB], bf16, isOutput=False)
    bproj = nc.declare_dram_parameter("b_proj", [C], f32, isOutput=False)
    ones_in = nc.declare_dram_parameter("ones64", [128, D], bf16, isOutput=False)
    y = nc.declare_dram_parameter("y", [N, C], bf16, isOutput=True)

    bproj_bcast = bass.AP(tensor=bproj, offset=0, ap=[[0, 128], [1, C]])

    with tile.TileContext(nc) as tc:
        with contextlib.ExitStack() as ctx:
            ep0 = ctx.enter_context
            consts = ep0(tc.tile_pool(name="consts", bufs=1))
            ones64 = consts.tile([128, D], bf16)
            nc.scalar.dma_start(out=ones64, in_=ones_in[:])
            ones1_f32 = consts.tile([1, D], f32)
            nc.vector.tensor_copy(ones1_f32, ones64[0:1, :])
            b_bc = consts.tile([128, C], f32)

            # pools living across reps (x double-buffered; weights resident)
            xT_p = ep0(tc.tile_pool(name="xT", bufs=2))
            wqk_p = ep0(tc.tile_pool(name="wqk", bufs=1))
            wv_p = ep0(tc.tile_pool(name="wv", bufs=1))
            wp_p = ep0(tc.tile_pool(name="wp", bufs=1))
            wqk_tiles = {}
            wvs = []
            wp = None

            for rep in range(n_rep):
              with contextlib.ExitStack() as rctx:
                ep = rctx.enter_context
                v_p = ep(tc.tile_pool(name=f"v_r{rep}", bufs=1))
                oT_p = ep(tc.tile_pool(name=f"oT_r{rep}", bufs=1))
                qk_p = ep(tc.tile_pool(name=f"qk_r{rep}", bufs=2))
                e2_p = ep(tc.tile_pool(name=f"e2_r{rep}", bufs=8))
                o65_p = ep(tc.tile_pool(name=f"o65_r{rep}", bufs=4))
                inv_p = ep(tc.tile_pool(name=f"inv_r{rep}", bufs=4))
                invd_p = ep(tc.tile_pool(name=f"invd_r{rep}", bufs=4, space="DRAM"))
                invb_p = ep(tc.tile_pool(name=f"invb_r{rep}", bufs=4))
                xTs = [
                    xT_p.tile([128, N], bf16, name=f"xT{c}_r{rep}", tag=f"xT{c}")
                    for c in range(PCHUNKS)
                ]
                v_flat = v_p.tile(
                    [128, PCHUNKS, H, D + 1], bf16, name=f"v_r{rep}", tag="v"
                )
                v_ext = [v_flat[:, t] for t in range(PCHUNKS)]
                oT = [
                    oT_p.tile([128, N], bf16, name=f"oT{i}_r{rep}", tag=f"oT{i}")
                    for i in range(HP)
                ]

                # x in: SP queue only (head work; prefetches across reps)
                for c in range(PCHUNKS):
                    nc.sync.dma_start(out=xTs[c], in_=xT[c * 128 : (c + 1) * 128, :])

                if rep == 0:
                    # resident weights: load once on the Act queue
                    for hp in range(HP):
                        wq = wqk_p.tile(
                            [128, PCHUNKS, 128], bf16, name=f"wq{hp}", tag=f"wq{hp}"
                        )
                        nc.scalar.dma_start(out=wq, in_=wq_pk[hp])
                        wk = wqk_p.tile(
                            [128, PCHUNKS, 128], bf16, name=f"wk{hp}", tag=f"wk{hp}"
                        )
                        nc.scalar.dma_start(out=wk, in_=wk_pk[hp])
                        wqk_tiles[hp] = (wq, wk)
                        if hp == 0:
                            for vb in range(2):
                                wv = wv_p.tile(
                                    [128, PCHUNKS, TB], bf16,
                                    name=f"wv{vb}", tag=f"wv{vb}",
                                )
                                nc.scalar.dma_start(out=wv, in_=wv_pk[vb])
                                wvs.append(wv)
                    wp = wp_p.tile(
                        [128, 2, PCHUNKS, TB], bf16, name="wp", tag="wp"
                    )
                    for half in range(2):
                        nc.sync.dma_start(out=wp[:, half], in_=wp_pk[half])
                    nc.sync.dma_start(out=b_bc, in_=bproj_bcast)

                for t in range(PCHUNKS):
                    nc.vector.tensor_copy(
                        v_ext[t][:, :, D : D + 1], ones64[:, 0:H, None]
                    )

                qk = {}

                # ---- qkv(0): chunk-major, paced by x-chunk arrival ----
                with tc.tile_pool(name=f"psq0_r{rep}", bufs=1, space="PSUM") as q0p:
                    wq0, wk0 = wqk_tiles[0]
                    accs = [
                        q0p.tile([128, TB], f32, name=f"acc{j}_r{rep}", tag=f"acc{j}")
                        for j in range(4)
                    ]
                    for c in range(PCHUNKS):
                        for qi, w in ((0, wq0), (1, wk0)):
                            for tb in range(2):
                                nc.tensor.matmul(
                                    accs[2 * qi + tb],
                                    w[:, c, :],
                                    xTs[c][:, tb * TB : (tb + 1) * TB],
                                    start=(c == 0),
                                    stop=(c == PCHUNKS - 1),
                                )
                    qk[0] = qk_p.tile([128, 2, N], bf16, name=f"qk0_r{rep}", tag="qk")
                    for qi in range(2):
                        for tb in range(2):
                            nc.vector.tensor_copy(
                                qk[0][:, qi, tb * TB : (tb + 1) * TB],
                                accs[2 * qi + tb],
                            )

                psst_p = ep(tc.tile_pool(name=f"psst_r{rep}", bufs=2, space="PSUM"))
                pso_p = ep(tc.tile_pool(name=f"pso_r{rep}", bufs=2, space="PSUM"))
                psq_p = ep(tc.tile_pool(name=f"psq_r{rep}", bufs=2, space="PSUM"))

                # Extra PE work (qkv for later head-pairs, v chains) is
                # emitted as single-matmul items with weights, drained a few
                # per attention slot so per-slot PE time stays below the act
                # period and the exp chain never starves. Accumulation-group
                # members may interleave with other-bank matmuls (the av
                # groups already rely on this).
                def accum_items(alloc, mm, copy):
                    cell = []
                    items = []
                    for c in range(PCHUNKS):
                        def run(c=c):
                            if c == 0:
                                cell.append(alloc())
                            mm(cell[0], c)
                        items.append((1, run))
                    items.append((0, lambda: copy(cell[0])))
                    return items

                def qkv_items(hp):
                    wq, wk = wqk_tiles[hp]
                    qk[hp] = qk_p.tile(
                        [128, 2, N], bf16, name=f"qk{hp}_r{rep}", tag="qk"
                    )
                    items = []
                    for tb in range(N // TB):
                        for qi, w in ((0, wq), (1, wk)):
                            tbs = slice(tb * TB, (tb + 1) * TB)
                            items += accum_items(
                                lambda: psq_p.tile(
                                    [128, TB], f32, name=f"pq_r{rep}", tag="pq"
                                ),
                                lambda p, c, w=w, tbs=tbs: nc.tensor.matmul(
                                    p, w[:, c, :], xTs[c][:, tbs],
                                    start=(c == 0), stop=(c == PCHUNKS - 1),
                                ),
                                lambda p, hp=hp, qi=qi, tbs=tbs:
                                    nc.vector.tensor_copy(qk[hp][:, qi, tbs], p),
                            )
                    return items

                def v_items(vb, t):
                    wv = wvs[vb]
                    return accum_items(
                        lambda: psq_p.tile(
                            [128, TB], f32, name=f"pv_r{rep}", tag="pq"
                        ),
                        lambda p, c: nc.tensor.matmul(
                            p,
                            xTs[c][:, t * 128 : (t + 1) * 128],
                            wv[:, c, :],
                            start=(c == 0), stop=(c == PCHUNKS - 1),
                        ),
                        lambda p: nc.vector.tensor_copy(
                            v_ext[t][:, vb * 8 : (vb + 1) * 8, 0:D],
                            p.rearrange("p (h d) -> p h d", h=8),
                        ),
                    )

                # ---------------- attention pipeline ----------------
                def S_pack(hp, hh, mb, tp):
                    """Two S^T chunks (2tp, 2tp+1) -> one 2-bank exp."""
                    qn = qk[hp]
                    b0 = 64 * hh
                    mbs = slice(mb * TB, (mb + 1) * TB)
                    e2 = e2_p.tile([128, 2, TB], bf16, name=f"e2_r{rep}", tag="e2")
                    ps2 = psst_p.tile(
                        [128, 2, TB], f32, name=f"ps2_r{rep}", tag="ps2"
                    )
                    for i in (0, 1):
                        t = 2 * tp + i
                        nc.tensor.matmul(
                            ps2[:, i, :],
                            qn[b0 : b0 + 64, 1, t * 128 : (t + 1) * 128],
                            qn[b0 : b0 + 64, 0, mbs],
                            start=True,
                            stop=True,
                        )
                    nc.scalar.activation(e2, ps2, Exp, scale=SCALE)
                    return e2

                def av_mms(po, hp, hh, e2t, tp):
                    h = 2 * hp + hh
                    for i in (0, 1):
                        t = 2 * tp + i
                        nc.tensor.matmul(
                            po,
                            v_ext[t][:, h, :],
                            e2t[:, i, :],
                            start=(t == 0),
                            stop=(t == PCHUNKS - 1),
                        )

                def av_norm(po, hp, hh, mb, pe_bcast=False):
                    """Copy PSUM out, reciprocal of sums, partition-broadcast
                    (DMA roundtrip, or a PE ones-broadcast when the result
                    gates the projection), normalize into oT."""
                    o65 = o65_p.tile([D + 1, TB], f32, name=f"o65_r{rep}", tag="o65")
                    nc.vector.tensor_copy(o65, po)
                    inv = inv_p.tile([1, TB], f32, name=f"inv_r{rep}", tag="inv")
                    nc.vector.reciprocal(inv, o65[D : D + 1, :])
                    if pe_bcast:
                        ibp = pso_p.tile([D, TB], f32, name=f"ibp_r{rep}", tag="po")
                        nc.tensor.matmul(ibp, ones1_f32, inv, start=True, stop=True)
                        nc.vector.tensor_mul(
                            oT[hp][hh * D : (hh + 1) * D, mb * TB : (mb + 1) * TB],
                            o65[0:D, :],
                            ibp,
                        )
                        return
                    dinv = invd_p.tile([1, TB], f32, name=f"dinv_r{rep}", tag="dinv")
                    nc.sync.dma_start(out=dinv, in_=inv)
                    ib = invb_p.tile([D, TB], f32, name=f"invb_r{rep}", tag="invb")
                    nc.sync.dma_start(
                        out=ib,
                        in_=bass.AP(
                            tensor=dinv.tensor,
                            offset=dinv.offset,
                            ap=[[0, D]] + list(dinv.ap)[1:],
                        ),
                    )
                    nc.vector.tensor_mul(
                        oT[hp][hh * D : (hh + 1) * D, mb * TB : (mb + 1) * TB],
                        o65[0:D, :],
                        ib,
                    )

                UNITS = [(hh, mb) for hh in range(2) for mb in range(2)]
                pending = None
                # extra PE work queues, drained a few matmuls per unit so the
                # act chain stays fed. qkv items are higher priority (due one
                # hp later); v chains have loose deadlines (vb0 before unit
                # a1's av, vb1 before hp4). force() guards the deadlines —
                # emitting a consumer before its producer items would
                # deadlock the in-order PE queue.
                q_qkv = []
                q_v = []
                for t in range(PCHUNKS):
                    q_v += v_items(0, t)

                def drain(budget):
                    used = 0
                    while q_qkv or q_v:
                        src = q_qkv if q_qkv else q_v
                        wgt, fn = src[0]
                        if used + wgt > budget:
                            break
                        src.pop(0)
                        fn()
                        used += wgt

                def force(q):
                    while q:
                        q.pop(0)[1]()

                for hp in range(HP):
                    force(q_qkv)  # qkv(hp) complete before S_pack(hp)
                    if hp == 1:
                        for t in range(PCHUNKS):
                            q_v += v_items(1, t)
                    if hp == 4:
                        force(q_v)  # vb1 complete before hp4's av
                    if hp + 1 < HP:
                        q_qkv.extend(qkv_items(hp + 1))
                    for hh, mb in UNITS:
                        if hp == 0 and (hh, mb) == (0, 1):
                            force(q_v)  # vb0 complete before unit a1's av
                        # batch same-shape matmuls into runs — alternating
                        # stationary shapes costs ~330ns per switch (measured)
                        po = pso_p.tile([D + 1, TB], f32, name=f"po_r{rep}", tag="po")
                        e2ts = []
                        for tp in (0, 1):
                            e2ts.append(S_pack(hp, hh, mb, tp))
                        if pending is not None:
                            for tp in range(PCHUNKS // 2):
                                av_mms(
                                    pending[0], pending[1], pending[2],
                                    pending[4][tp], tp,
                                )
                        drain(44 if pending is None else 16)
                        for tp in (2, 3):
                            e2ts.append(S_pack(hp, hh, mb, tp))
                        if pending is None:
                            drain(28)
                        if pending is not None:
                            av_norm(
                                pending[0], pending[1], pending[2], pending[3],
                                pe_bcast=(pending[1] == HP - 1 and pending[2] == 1),
                            )
                        pending = (po, hp, hh, mb, e2ts)

                # drain last unit
                po, hp, hh, mb, e2ts = pending
                for tp in range(PCHUNKS // 2):
                    av_mms(po, hp, hh, e2ts[tp], tp)
                av_norm(po, hp, hh, mb, pe_bcast=True)

                # ---------------- output projection ----------------
                # py tiles reuse the psq pool (same bank shape) — avoids a
                # PSUM pool open/close barrier between attention and proj.
                with tc.tile_pool(name=f"ysb_r{rep}", bufs=3) as ysb_p:
                    # token blocks 0-3 first: their oT inputs are ready well
                    # before the last unit's (blocks 4-7) normalize lands.
                    for mc_grp in (range(0, 4), range(4, 8)):
                        for cb in range(C // TB):
                            for mc in mc_grp:
                                py = psq_p.tile(
                                    [128, TB], f32, name=f"py_r{rep}", tag="pq"
                                )
                                for hp_ in range(HP):
                                    nc.tensor.matmul(
                                        py,
                                        oT[hp_][:, mc * 128 : (mc + 1) * 128],
                                        wp[:, cb, hp_, :],
                                        start=(hp_ == 0),
                                        stop=(hp_ == HP - 1),
                                    )
                                ys = ysb_p.tile(
                                    [128, TB], bf16, name=f"ys_r{rep}", tag="ys"
                                )
                                nc.vector.tensor_add(
                                    ys, py, b_bc[:, cb * TB : (cb + 1) * TB]
                                )
                                nc.scalar.dma_start(
                                    out=y[
                                        mc * 128 : (mc + 1) * 128,
                                        cb * TB : (cb + 1) * TB,
                                    ],
                                    in_=ys,
                                )

    _split_excess_waits(nc)
    nc.finalize()
    return nc


def _get_nc(n_rep=1, **opts):
    key = f"nc{n_rep}{sorted(opts.items())}"
    if key not in _cached:
        _cached[key] = _build(n_rep, **opts)
    return _cached[key]


def make_in_maps(x, W_qkv, W_proj, b_proj):
    import ml_dtypes

    bf16 = ml_dtypes.bfloat16
    x = np.asarray(x, dtype=np.float32)
    W_qkv = np.asarray(W_qkv, dtype=np.float32).astype(bf16)
    W_proj = np.asarray(W_proj, dtype=np.float32).astype(bf16)
    b_proj = np.ascontiguousarray(np.asarray(b_proj, dtype=np.float32))
    ones64 = np.ones((128, D), dtype=bf16)

    def pack(w, col0, ncol, nblk):
        # [C, ncol] cols [col0:col0+ncol) per block -> [nblk, 128, 8, ncol]
        out = np.empty((nblk, 128, PCHUNKS, ncol), dtype=bf16)
        for b in range(nblk):
            wb = w[:, col0 + b * ncol : col0 + (b + 1) * ncol]
            out[b] = wb.reshape(PCHUNKS, 128, ncol).transpose(1, 0, 2)
        return np.ascontiguousarray(out)

    wq_pk = pack(W_qkv, 0, 128, HP)
    wk_pk = pack(W_qkv, C, 128, HP)
    wv_pk = pack(W_qkv, 2 * C, TB, 2)
    wp_pk = pack(W_proj, 0, TB, 2)
    return [
        {
            "xT": np.ascontiguousarray(x[c].T.astype(bf16)),
            "wq_pk": wq_pk,
            "wk_pk": wk_pk,
            "wv_pk": wv_pk,
            "wp_pk": wp_pk,
            "b_proj": b_proj,
            "ones64": ones64,
        }
        for c in range(NCORES)
    ]


def kernel(x, W_qkv, W_proj, b_proj, **_ignored):
    import time as _time

    from concourse.bass_utils import run_bass_kernel_spmd

    nc = _get_nc()
    in_maps = make_in_maps(x, W_qkv, W_proj, b_proj)
    # transient device/tunnel errors (NRT_EXEC_UNIT_UNRECOVERABLE, INTERNAL
    # RPC hiccups) recover on re-dispatch; retry with backoff
    last_err = None
    for attempt in range(4):
        try:
            res = run_bass_kernel_spmd(nc, in_maps, core_ids=list(range(NCORES)))
            break
        except Exception as e:
            last_err = e
            _time.sleep(5 * (attempt + 1))
    else:
        raise last_err
    out = np.stack([res.results[c]["y"] for c in range(NCORES)], axis=0)
    return out.astype(np.float32)
